# revision 15
# baseline (speedup 1.0000x reference)
"""AttentionHead kernel for 8 Trainium2 NeuronCores.

Problem (per sample, B=4): x:[256,64,64] -> q/k/v 1x1-conv projections
(+positional encoding on q,k), S = q^T k / 8, softmax over the QUERY axis,
out = attn @ v, then 1x1-conv MLP with Mish + residual.

Sharding: 2 cores per sample, split over the query axis i (2048 queries each).
Softmax normalizes over i, so the per-key denominator den[j] = sum_i exp(S[i,j])
needs one tiny AllReduce per core pair (done in 4 chunks, latency hidden);
den folds into v (v/den), everything else is local, output halves disjoint.

v2 restructure vs the phase-separated baseline: the ScalarE exp stream
(~72us for 32x [128,2048] Exp activations) is the pacing resource, so the
attn@v matmuls are interleaved INTO the exp stream instead of running as a
separate phase afterwards.  PSUM: one [128,2048] S tile (4 banks) + three
[128,512] slots (3 banks) shared by attn@v chunk-accumulation and all
projection/MLP matmuls.  attn@v accumulates per 8-key-tile chunk in PSUM and
is folded into an SBUF fp32 accumulator by DVE adds, so chunk c's matmuls run
while chunk c+1's S tiles are still being exp'd.  This keeps the PE busy
end-to-end (no >3.4us idle -> no HAM re-throttle to 1.2GHz) and removes the
55us serialized attn@v phase.

Layout trick (unchanged): compute S transposed, S[j,i] = (k^T q)[j,i], keys j
on partitions.  exp runs PSUM->SBUF with a per-partition accumulate (the
denominator for free), and exp(S)[j,i] is directly the correct operand layout
for both out[c,i] = sum_j v[c,j]*attnT[j,i] and the MLP - zero transposes.
All matmul operands bf16 (fp32 PSUM accumulation).

MLP uses the hardware Mish LUT: h = Mish(psum + b1) in one activation.
Biases: q/k biases fold into the positional-encoding tensors on the host; the
v bias is added during the PSUM->SBUF move; b2 rides the final residual add.
"""

import numpy as np
import ml_dtypes

import concourse.bass as bass
import concourse.bacc as bacc
import concourse.mybir as mybir
import concourse.tile as tile

BF16 = mybir.dt.bfloat16
F32 = mybir.dt.float32
AF = mybir.ActivationFunctionType
OP = mybir.AluOpType
bf16 = ml_dtypes.bfloat16

B, C, H, W = 4, 256, 64, 64
N = H * W            # 4096 pixels
QK = 64
IS = N // 2          # 2048 queries per core
NJT = N // 128       # 32 key tiles
NIB = IS // 512      # 4 i-blocks
NCH = 4              # den allreduce chunks
JCH = NJT // NCH     # 8 key tiles per chunk
N_CORES = 8
REPLICA_GROUPS = [[0, 1], [2, 3], [4, 5], [6, 7]]


def build_program(n_cores: int = N_CORES, enable_asserts: bool = False,
                  use_mish: bool = True) -> bass.Bass:
    nc = bacc.Bacc(
        "TRN2",
        target_bir_lowering=False,
        debug=False,
        enable_asserts=enable_asserts,
        num_devices=n_cores,
    )

    # Per-core inputs (data differs by core; program is identical).
    # xq: q-projection input, ib-interleaved: [:, ib*1024+kt*512+c] is channel
    #   half kt, query block ib.
    # xb: k/v-projection input, column-block-interleaved: [:, jb*1024+kt*512+c]
    #   is channel half kt, pixel columns jb*512+c (both halves of a 512-col
    #   block adjacent, so one DMA chunk unlocks k_proj(jb)).
    xq_d = nc.dram_tensor("xq", [128, 2 * IS], BF16, kind="ExternalInput").ap()
    xb_d = nc.dram_tensor("xb", [128, 2 * N], BF16, kind="ExternalInput").ap()
    xf_d = nc.dram_tensor("xf", [128, 2 * IS], BF16, kind="ExternalInput").ap()
    pe1q_d = nc.dram_tensor("pe1q", [QK, IS], BF16, kind="ExternalInput").ap()
    # Shared weights (same on all cores).
    pe1_d = nc.dram_tensor("pe1", [QK, N], BF16, kind="ExternalInput").ap()
    wqk_d = nc.dram_tensor("wqk", [128, 256], BF16, kind="ExternalInput").ap()
    # wmlp = wvt | w1t | w2t | bvb
    wmlp_d = nc.dram_tensor("wmlp", [128, 1792], BF16, kind="ExternalInput").ap()
    bcols_d = nc.dram_tensor("bcols", [128, 4], F32, kind="ExternalInput").ap()

    y_d = nc.dram_tensor("y", [C, IS], F32, kind="ExternalOutput").ap()

    with tile.TileContext(nc) as tc:
        with (
            tc.tile_pool(name="const", bufs=1) as cpool,
            tc.tile_pool(name="qk", bufs=1) as qkpool,
            tc.tile_pool(name="den", bufs=1) as denpool,
            tc.tile_pool(name="dram", bufs=1, space="DRAM") as dram,
            tc.tile_pool(name="psS", bufs=1, space="PSUM") as psS,
            tc.tile_pool(name="psO", bufs=3, space="PSUM") as psO,
            tc.tile_pool(name="psP", bufs=1, space="PSUM") as psP,
        ):
            # q-projection inputs first (gate the whole pipeline), then the
            # rest; bulky non-urgent loads go on the gpsimd queue.
            wqk_sb = cpool.tile([128, 256], BF16)
            nc.sync.dma_start(wqk_sb[:], wqk_d[:])

            q_sb = qkpool.tile([QK, IS], BF16)     # q, d on partitions
            k_sb = qkpool.tile([QK, N], BF16)      # k, d on partitions
            vtpool = tc.alloc_tile_pool(name="vt", bufs=1, side="right")
            vt_sb = vtpool.tile([128, NJT * 256], BF16)   # v^T, j on partitions
            den_sb = denpool.tile([128, NJT], F32)
            dsum_sb = denpool.tile([128, NJT], F32)
            rden_sb = denpool.tile([128, NJT], F32)

            with tc.tile_pool(name="attn", bufs=1) as apool:
                attn_sb = apool.tile([128, NJT * IS], BF16)   # 16 MiB
                # staging pools, allocated in reverse order of death so the
                # stack allocator can pop them: pe1q/xq/pe1 die by jt7,
                # xb by jt10 (its region becomes the fp32 out-accumulators).
                xbpool = tc.alloc_tile_pool(name="xb", bufs=1)
                pe1pool = tc.alloc_tile_pool(name="pe1", bufs=1)
                xqpool = tc.alloc_tile_pool(name="xq", bufs=1)
                pe1qpool = tc.alloc_tile_pool(name="pe1q", bufs=1)
                xb_sb = xbpool.tile([128, 2 * N], BF16)
                xq_sb = xqpool.tile([128, 2 * IS], BF16)
                pe1q_sb = pe1qpool.tile([QK, IS], BF16)
                pe1_sb = pe1pool.tile([QK, N], BF16)
                for ch in range(2):
                    nc.sync.dma_start(xq_sb[:, bass.ts(ch, IS)],
                                      xq_d[:, bass.ts(ch, IS)])
                nc.sync.dma_start(pe1q_sb[:], pe1q_d[:])
                for ch in range(4):
                    nc.sync.dma_start(xb_sb[:, bass.ts(ch, N // 2)],
                                      xb_d[:, bass.ts(ch, N // 2)])
                for ch in range(2):
                    nc.sync.dma_start(pe1_sb[:, bass.ts(ch, N // 2)],
                                      pe1_d[:, bass.ts(ch, N // 2)])
                wmlp_sb = cpool.tile([128, 1792], BF16)
                bcols_sb = cpool.tile([128, 4], F32)
                nc.gpsimd.dma_start(wmlp_sb[:], wmlp_d[:])
                nc.gpsimd.dma_start(bcols_sb[:], bcols_d[:])
                wvt = wmlp_sb[:, 0:512]
                w1t = wmlp_sb[:, 512:1024]
                w2t = wmlp_sb[:, 1024:1536]
                bvb = wmlp_sb[:, 1536:1792]
                b1c = bcols_sb[:, 0:2]
                b2c = bcols_sb[:, 2:4]

                # lazily allocated (regions recycled from released staging)
                vts_cell = [None]     # v^T / den (right side, after jt7)
                oacc = [None, None]   # fp32 attn-out accumulators (ex-xb)

                def q_proj(ib):
                    sl = bass.ts(ib, 512)
                    ps = psP.tile([128, 512], F32, name="psp")[0:QK, :]
                    for kt in range(2):
                        nc.tensor.matmul(ps[:], wqk_sb[:, bass.ts(kt, QK)],
                                         xq_sb[:, ib * 1024 + kt * 512:
                                               ib * 1024 + (kt + 1) * 512],
                                         start=(kt == 0), stop=(kt == 1))
                    nc.vector.tensor_add(q_sb[:, sl], ps[:], pe1q_sb[:, sl])

                def k_proj(jb):
                    sl = bass.ts(jb, 512)
                    ps = psP.tile([128, 512], F32, name="psp")[0:QK, :]
                    for kt in range(2):
                        nc.tensor.matmul(ps[:], wqk_sb[:, 128 + kt * QK:
                                                       128 + (kt + 1) * QK],
                                         xb_sb[:, jb * 1024 + kt * 512:
                                               jb * 1024 + (kt + 1) * 512],
                                         start=(kt == 0), stop=(kt == 1))
                    nc.vector.tensor_add(k_sb[:, sl], ps[:], pe1_sb[:, sl])

                def v_proj(jt):
                    jb, jr = jt // 4, jt % 4
                    ps = psP.tile([128, 512], F32, name="psp")[:, 0:256]
                    for kt in range(2):
                        nc.tensor.matmul(ps[:],
                                         xb_sb[:, jb * 1024 + kt * 512 + jr * 128:
                                               jb * 1024 + kt * 512 + (jr + 1) * 128],
                                         wvt[:, bass.ts(kt, 256)],
                                         start=(kt == 0), stop=(kt == 1))
                    nc.vector.tensor_add(vt_sb[:, bass.ts(jt, 256)],
                                         ps[:], bvb[:])

                def s_tile(jt):
                    ps = psS.tile([128, IS], F32, name="pss")
                    for ib in range(NIB):
                        nc.tensor.matmul(ps[:, bass.ts(ib, 512)],
                                         k_sb[:, bass.ts(jt, 128)],
                                         q_sb[:, bass.ts(ib, 512)],
                                         start=True, stop=True)
                    nc.scalar.activation(attn_sb[:, bass.ts(jt, IS)], ps[:],
                                         AF.Exp, scale=0.125,
                                         accum_out=den_sb[:, jt:jt + 1])

                def den_chunk(ch):
                    csl = bass.ts(ch, JCH)
                    den_in = dram.tile([128, JCH], F32, name=f"den_in{ch}")
                    den_out = dram.tile([128, JCH], F32, name=f"den_out{ch}")
                    nc.sync.dma_start(den_in[:], den_sb[:, csl])
                    nc.gpsimd.collective_compute(
                        "AllReduce", OP.add,
                        replica_groups=REPLICA_GROUPS,
                        ins=[den_in.opt()], outs=[den_out.opt()],
                    )
                    nc.sync.dma_start(dsum_sb[:, csl], den_out[:])
                    nc.vector.reciprocal(rden_sb[:, csl], dsum_sb[:, csl])
                    vts_sb = vts_cell[0]
                    for jt in range(ch * JCH, (ch + 1) * JCH):
                        nc.vector.tensor_scalar_mul(vts_sb[:, bass.ts(jt, 256)],
                                                    vt_sb[:, bass.ts(jt, 256)],
                                                    rden_sb[:, jt:jt + 1])

                def av_group(ch, mt, ib):
                    """attn@v partial over chunk ch's key tiles, folded into
                    the fp32 SBUF accumulator by a DVE add."""
                    ps = psO.tile([128, 512], F32, name="psav")
                    vts_sb = vts_cell[0]
                    for jt in range(ch * JCH, (ch + 1) * JCH):
                        nc.tensor.matmul(
                            ps[:],
                            vts_sb[:, jt * 256 + mt * 128:
                                   jt * 256 + (mt + 1) * 128],
                            attn_sb[:, jt * IS + ib * 512:
                                    jt * IS + (ib + 1) * 512],
                            start=(jt == ch * JCH), stop=(jt == (ch + 1) * JCH - 1),
                            skip_group_check=True)
                    sl = bass.ts(ib, 512)
                    if ch == 0:
                        nc.vector.tensor_copy(oacc[mt][:, sl], ps[:])
                    else:
                        nc.vector.tensor_add(oacc[mt][:, sl], ps[:],
                                             oacc[mt][:, sl])

                # ---- emission schedule ----
                for ib in range(NIB):
                    q_proj(ib)
                k_proj(0)

                # av groups become safe ~3 key tiles after their chunk's
                # allreduce is issued.
                av_sched = {}
                for ch in range(NCH):
                    for g, (mt, ib) in enumerate(
                            (m, i) for i in range(NIB) for m in range(2)):
                        jt_emit = ch * JCH + JCH + 3 + g
                        av_sched.setdefault(jt_emit, []).append((ch, mt, ib))

                post = []   # groups that fall past the jt stream
                for jt in range(NJT):
                    s_tile(jt)
                    # keep the PE fed while ScalarE runs exp(jt): projections
                    # first, then ready av groups.
                    jb = jt + 1
                    if jb < N // 512:
                        k_proj(jb)
                    for vj in range(3 * jt, min(3 * jt + 3, NJT)):
                        v_proj(vj)
                    if jt == 7:
                        pe1qpool.release()
                        xqpool.release()
                        pe1pool.release()
                        vtspool = tc.alloc_tile_pool(name="vts", bufs=1,
                                                     side="right")
                        vts_cell[0] = vtspool.tile([128, NJT * 256], BF16,
                                                   name="vts_sb")
                    if jt == 10:
                        xbpool.release()
                        opool = tc.alloc_tile_pool(name="oacc", bufs=1)
                        for m in range(2):
                            oacc[m] = opool.tile([128, IS], F32,
                                                 name=f"oacc{m}")
                    if jt % JCH == JCH - 1:
                        den_chunk(jt // JCH)
                    for item in av_sched.pop(jt, []):
                        if jt < NJT - 1:
                            av_group(*item)
                        else:
                            post.append(item)
                for jts in sorted(av_sched):
                    post.extend(av_sched[jts])

                # remaining attn@v groups, then drop vts/vt for the MLP pools
                post.sort(key=lambda t: (t[0], t[2], t[1]))
                for item in post:
                    av_group(*item)
                vtspool.release()
                vtpool.release()

                # ---- MLP tail ----
                # h = mish(W1 out + b1) via the exact identity
                #   mish(x) = x * t1 * r,  t1 = (e+2)*e,  r = 1/(t1+2),  e = e^x
                # (one Exp on ScalarE - same table as the attention stream,
                # zero table switches - plus a 4-op DVE chain; DVE reciprocal
                # needs no LUT).  y = W2 h + b2 + x.
                with (
                    tc.tile_pool(name="outb", bufs=1) as outpool,
                    tc.tile_pool(name="xf", bufs=1) as xfpool,
                    tc.tile_pool(name="h", bufs=1) as hpool,
                    tc.tile_pool(name="mt", bufs=3) as mpool,
                    tc.tile_pool(name="y", bufs=2) as ypool,
                ):
                    out_sb = [outpool.tile([128, IS], BF16, name=f"out{m}")
                              for m in range(2)]
                    for mt in range(2):
                        for ib in range(NIB):
                            # ScalarE Identity copy: DVE is busy with the
                            # final accumulator adds at this point.
                            nc.scalar.activation(
                                out_sb[mt][:, bass.ts(ib, 512)],
                                oacc[mt][:, bass.ts(ib, 512)], AF.Identity)
                    xf_sb = xfpool.tile([128, 2 * IS], BF16)
                    nc.gpsimd.dma_start(xf_sb[:], xf_d[:])
                    h_sb = [hpool.tile([128, IS], BF16, name=f"h_sb{m}")
                            for m in range(2)]

                    for ib in range(NIB):
                        sl = bass.ts(ib, 512)
                        for mt in range(2):
                            ps = psO.tile([128, 512], F32, name="psav")
                            for kt in range(2):
                                nc.tensor.matmul(
                                    ps[:],
                                    w1t[:, kt * 256 + mt * 128:
                                        kt * 256 + (mt + 1) * 128],
                                    out_sb[kt][:, sl],
                                    start=(kt == 0), stop=(kt == 1),
                                    skip_group_check=True)
                            e_t = mpool.tile([128, 512], BF16, name="mish_e")
                            t1_t = mpool.tile([128, 512], BF16, name="mish_t1")
                            r_t = mpool.tile([128, 512], BF16, name="mish_r")
                            m_t = mpool.tile([128, 512], BF16, name="mish_m")
                            nc.scalar.activation(e_t[:], ps[:], AF.Exp,
                                                 bias=b1c[:, mt:mt + 1])
                            nc.vector.scalar_tensor_tensor(
                                t1_t[:], e_t[:], 2.0, e_t[:],
                                op0=OP.add, op1=OP.mult)
                            nc.vector.tensor_scalar_add(r_t[:], t1_t[:], 2.0)
                            with nc.allow_low_precision(
                                    reason="mish: bf16 reciprocal, error "
                                           "averages out across 256 ch"):
                                nc.vector.reciprocal(r_t[:], r_t[:])
                            nc.vector.tensor_mul(m_t[:], t1_t[:], r_t[:])
                            nc.vector.scalar_tensor_tensor(
                                h_sb[mt][:, sl], ps[:], b1c[:, mt:mt + 1],
                                m_t[:], op0=OP.add, op1=OP.mult)
                    for ib in range(NIB):
                        sl = bass.ts(ib, 512)
                        for mt in range(2):
                            ps = psO.tile([128, 512], F32, name="psav")
                            for kt in range(2):
                                nc.tensor.matmul(
                                    ps[:],
                                    w2t[:, kt * 256 + mt * 128:
                                        kt * 256 + (mt + 1) * 128],
                                    h_sb[kt][:, sl],
                                    start=(kt == 0), stop=(kt == 1),
                                    skip_group_check=True)
                            y_sb = ypool.tile([128, 512], F32)
                            nc.vector.scalar_tensor_tensor(
                                y_sb[:], ps[:], b2c[:, mt:mt + 1],
                                xf_sb[:, mt * IS + ib * 512:
                                      mt * IS + (ib + 1) * 512],
                                op0=OP.add, op1=OP.add)
                            nc.sync.dma_start(
                                y_d[mt * 128:(mt + 1) * 128, sl], y_sb[:])
                opool.release()
    nc.finalize()
    return nc


def _to_lhsT_sb(w):
    """[256, M] fp32 -> SBUF layout [128, 2*M] bf16: col block kt holds rows
    kt*128..kt*128+127 of w."""
    k, m = w.shape
    assert k == 256
    return np.ascontiguousarray(
        w.reshape(2, 128, m).transpose(1, 0, 2).reshape(128, 2 * m).astype(bf16))


def _bf(a):
    return np.ascontiguousarray(np.asarray(a, dtype=np.float32).astype(bf16))


def _halves(a):
    """[256, X] -> [128, 2*X] with the two 128-row halves side by side."""
    return np.ascontiguousarray(np.concatenate([a[:128], a[128:]], axis=1))


def _blocks(a, blk=512):
    """[256, X] -> [128, 2*X] with 512-col blocks: block jb holds
    [rows 0-127 | rows 128-255] of cols jb*512..+512."""
    k, x = a.shape
    nb = x // blk
    out = np.empty((128, 2 * x), a.dtype)
    for jb in range(nb):
        out[:, jb * 2 * blk:jb * 2 * blk + blk] = a[:128, jb * blk:(jb + 1) * blk]
        out[:, jb * 2 * blk + blk:(jb + 1) * 2 * blk] = a[128:, jb * blk:(jb + 1) * blk]
    return np.ascontiguousarray(out)


def make_in_maps(x, WQ, bQ, WK, bK, WV, bV, PE, W1, b1, W2, b2, n_cores=N_CORES):
    x = np.asarray(x, dtype=np.float32)
    xf3 = np.ascontiguousarray(x.reshape(B, C, N))
    pef = np.asarray(PE, dtype=np.float32).reshape(QK, N)
    pe1 = _bf(pef + np.asarray(bK, np.float32)[:, None])
    pe1q_full = _bf(pef + np.asarray(bQ, np.float32)[:, None])

    wq = _to_lhsT_sb(np.asarray(WQ, np.float32).T)   # [128, 128]
    wk = _to_lhsT_sb(np.asarray(WK, np.float32).T)
    wmlp = np.concatenate([
        _to_lhsT_sb(np.asarray(WV, np.float32).T),
        _to_lhsT_sb(np.asarray(W1, np.float32).T),
        _to_lhsT_sb(np.asarray(W2, np.float32).T),
        np.broadcast_to(_bf(np.asarray(bV)[None, :]), (128, 256)),
    ], axis=1)
    bcols = np.concatenate([
        np.asarray(b1, np.float32).reshape(2, 128).T,
        np.asarray(b2, np.float32).reshape(2, 128).T,
    ], axis=1)

    shared = {
        "pe1": pe1,
        "wqk": np.ascontiguousarray(np.concatenate([wq, wk], axis=1)),
        "wmlp": np.ascontiguousarray(wmlp),
        "bcols": np.ascontiguousarray(bcols),
    }
    in_maps = []
    for core in range(n_cores):
        s, h = core // 2, core % 2
        isl = slice(h * IS, (h + 1) * IS)
        xb = _bf(xf3[s])
        m = dict(shared)
        m["xb"] = _blocks(xb)
        # xq is ib-interleaved: [:, ib*1024+kt*512 : ...] = channel-half kt,
        # query block ib - so the first DMA chunk covers ib 0-1 completely.
        xqs = xb[:, isl]
        m["xq"] = np.ascontiguousarray(np.concatenate(
            [np.concatenate([xqs[:128, ib * 512:(ib + 1) * 512],
                             xqs[128:, ib * 512:(ib + 1) * 512]], axis=1)
             for ib in range(NIB)], axis=1))
        m["xf"] = _halves(_bf(xf3[s][:, isl]))
        m["pe1q"] = np.ascontiguousarray(pe1q_full[:, isl])
        in_maps.append(m)
    return in_maps


def assemble_output(results, n_cores=N_CORES):
    y = np.empty((B, C, N), dtype=np.float32)
    for s in range(B):
        y[s][:, :IS] = results[2 * s]["y"]
        y[s][:, IS:] = results[2 * s + 1]["y"]
    return y.reshape(B, C, H, W)


_PROG = None


def kernel(**inputs) -> np.ndarray:
    global _PROG
    from concourse.bass_utils import run_bass_kernel_spmd
    if _PROG is None:
        _PROG = build_program(N_CORES)
    in_maps = make_in_maps(**inputs)
    res = run_bass_kernel_spmd(_PROG, in_maps, core_ids=list(range(N_CORES)))
    return assemble_output(res.results)


# revision 17
# speedup vs baseline: 1.0200x; 1.0200x over previous
"""AttentionHead kernel for 8 Trainium2 NeuronCores.

Problem (per sample, B=4): x:[256,64,64] -> q/k/v 1x1-conv projections
(+positional encoding on q,k), S = q^T k / 8, softmax over the QUERY axis,
out = attn @ v, then 1x1-conv MLP with Mish + residual.

Sharding: 2 cores per sample, split over the query axis i (2048 queries each).
Softmax normalizes over i, so the per-key denominator den[j] = sum_i exp(S[i,j])
needs one tiny AllReduce per core pair (done in 4 chunks, latency hidden);
den folds into v (v/den), everything else is local, output halves disjoint.

v2 restructure vs the phase-separated baseline: the ScalarE exp stream
(~72us for 32x [128,2048] Exp activations) is the pacing resource, so the
attn@v matmuls are interleaved INTO the exp stream instead of running as a
separate phase afterwards.  PSUM: one [128,2048] S tile (4 banks) + three
[128,512] slots (3 banks) shared by attn@v chunk-accumulation and all
projection/MLP matmuls.  attn@v accumulates per 8-key-tile chunk in PSUM and
is folded into an SBUF fp32 accumulator by DVE adds, so chunk c's matmuls run
while chunk c+1's S tiles are still being exp'd.  This keeps the PE busy
end-to-end (no >3.4us idle -> no HAM re-throttle to 1.2GHz) and removes the
55us serialized attn@v phase.

Layout trick (unchanged): compute S transposed, S[j,i] = (k^T q)[j,i], keys j
on partitions.  exp runs PSUM->SBUF with a per-partition accumulate (the
denominator for free), and exp(S)[j,i] is directly the correct operand layout
for both out[c,i] = sum_j v[c,j]*attnT[j,i] and the MLP - zero transposes.
All matmul operands bf16 (fp32 PSUM accumulation).

MLP uses the hardware Mish LUT: h = Mish(psum + b1) in one activation.
Biases: q/k biases fold into the positional-encoding tensors on the host; the
v bias is added during the PSUM->SBUF move; b2 rides the final residual add.
"""

import numpy as np
import ml_dtypes

import concourse.bass as bass
import concourse.bacc as bacc
import concourse.mybir as mybir
import concourse.tile as tile

BF16 = mybir.dt.bfloat16
F32 = mybir.dt.float32
AF = mybir.ActivationFunctionType
OP = mybir.AluOpType
bf16 = ml_dtypes.bfloat16

B, C, H, W = 4, 256, 64, 64
N = H * W            # 4096 pixels
QK = 64
IS = N // 2          # 2048 queries per core
NJT = N // 128       # 32 key tiles
NIB = IS // 512      # 4 i-blocks
NCH = 4              # den allreduce chunks
JCH = NJT // NCH     # 8 key tiles per chunk
N_CORES = 8
REPLICA_GROUPS = [[0, 1], [2, 3], [4, 5], [6, 7]]


def build_program(n_cores: int = N_CORES, enable_asserts: bool = False) -> bass.Bass:
    nc = bacc.Bacc(
        "TRN2",
        target_bir_lowering=False,
        debug=False,
        enable_asserts=enable_asserts,
        num_devices=n_cores,
    )

    # Per-core inputs (data differs by core; program is identical).
    # xq: q-projection input, ib-interleaved: [:, ib*1024+kt*512+c] is channel
    #   half kt, query block ib.
    # xb: k/v-projection input, column-block-interleaved: [:, jb*1024+kt*512+c]
    #   is channel half kt, pixel columns jb*512+c.
    xq_d = nc.dram_tensor("xq", [128, 2 * IS], BF16, kind="ExternalInput").ap()
    xb_d = nc.dram_tensor("xb", [128, 2 * N], BF16, kind="ExternalInput").ap()
    xf_d = nc.dram_tensor("xf", [128, 2 * IS], BF16, kind="ExternalInput").ap()
    pe1q_d = nc.dram_tensor("pe1q", [QK, IS], BF16, kind="ExternalInput").ap()
    # Shared weights (same on all cores).
    pe1_d = nc.dram_tensor("pe1", [QK, N], BF16, kind="ExternalInput").ap()
    wqk_d = nc.dram_tensor("wqk", [128, 256], BF16, kind="ExternalInput").ap()
    # wmlp = wvt | w1t | w2t | bvb
    wmlp_d = nc.dram_tensor("wmlp", [128, 1792], BF16, kind="ExternalInput").ap()
    bcols_d = nc.dram_tensor("bcols", [128, 4], F32, kind="ExternalInput").ap()

    y_d = nc.dram_tensor("y", [C, IS], F32, kind="ExternalOutput").ap()

    # den-allreduce chunks (key-tile ranges).  Front chunks are small so the
    # first attn@v groups unlock early and fill the PE during the exp stream.
    CHUNKS = [(0, 2), (2, 4), (4, 8), (8, 16), (16, 24), (24, 32)]

    with tile.TileContext(nc) as tc:
        with (
            tc.tile_pool(name="const", bufs=1) as cpool,
            tc.tile_pool(name="qk", bufs=1) as qkpool,
            tc.tile_pool(name="den", bufs=1) as denpool,
            tc.tile_pool(name="dram", bufs=1, space="DRAM") as dram,
            tc.tile_pool(name="psS", bufs=2, space="PSUM") as psS,
            tc.tile_pool(name="psO", bufs=3, space="PSUM") as psO,
            tc.tile_pool(name="psP", bufs=1, space="PSUM") as psP,
        ):
            wqk_sb = cpool.tile([128, 256], BF16)
            nc.sync.dma_start(wqk_sb[:], wqk_d[:])

            q_sb = qkpool.tile([QK, IS], BF16)     # q, d on partitions
            k_sb = qkpool.tile([QK, N], BF16)      # k, d on partitions
            den_sb = denpool.tile([128, NJT], F32)
            dsum_sb = denpool.tile([128, NJT], F32)
            rden_sb = denpool.tile([128, NJT], F32)
            den_h = denpool.tile([128, 2 * NJT], F32)   # per-half accums
            ones_sb = cpool.tile([1, 128], BF16)
            nc.vector.memset(ones_sb[:], 1.0)

            with tc.tile_pool(name="attn", bufs=1) as apool:
                attn_sb = apool.tile([128, NJT * IS], BF16)   # 16 MiB
                xbpool = tc.alloc_tile_pool(name="xb", bufs=1)
                pe1pool = tc.alloc_tile_pool(name="pe1", bufs=1)
                xqpool = tc.alloc_tile_pool(name="xq", bufs=1)
                pe1qpool = tc.alloc_tile_pool(name="pe1q", bufs=1)
                xb_sb = xbpool.tile([128, 2 * N], BF16)
                xq_sb = xqpool.tile([128, 2 * IS], BF16)
                pe1q_sb = pe1qpool.tile([QK, IS], BF16)
                pe1_sb = pe1pool.tile([QK, N], BF16)
                for ch in range(2):
                    nc.sync.dma_start(xq_sb[:, bass.ts(ch, IS)],
                                      xq_d[:, bass.ts(ch, IS)])
                nc.sync.dma_start(pe1q_sb[:], pe1q_d[:])
                for ch in range(4):
                    nc.sync.dma_start(xb_sb[:, bass.ts(ch, N // 2)],
                                      xb_d[:, bass.ts(ch, N // 2)])
                for ch in range(2):
                    nc.sync.dma_start(pe1_sb[:, bass.ts(ch, N // 2)],
                                      pe1_d[:, bass.ts(ch, N // 2)])
                wmlp_sb = cpool.tile([128, 1792], BF16)
                bcols_sb = cpool.tile([128, 4], F32)
                nc.gpsimd.dma_start(wmlp_sb[:], wmlp_d[:])
                nc.gpsimd.dma_start(bcols_sb[:], bcols_d[:])
                wvt = wmlp_sb[:, 0:512]
                w1t = wmlp_sb[:, 512:1024]
                w2t = wmlp_sb[:, 1024:1536]
                bvb = wmlp_sb[:, 1536:1792]
                b1c = bcols_sb[:, 0:2]
                b2c = bcols_sb[:, 2:4]

                # lazily allocated, on the right stack
                vts_cell = [None]     # v^T/den, [128, NJT*256] bf16
                oacc = [None, None]   # fp32 attn-out accumulators

                def q_proj(ib):
                    sl = bass.ts(ib, 512)
                    ps = psP.tile([128, 512], F32, name="psp")[0:QK, :]
                    for kt in range(2):
                        nc.tensor.matmul(ps[:], wqk_sb[:, bass.ts(kt, QK)],
                                         xq_sb[:, ib * 1024 + kt * 512:
                                               ib * 1024 + (kt + 1) * 512],
                                         start=(kt == 0), stop=(kt == 1))
                    nc.vector.tensor_add(q_sb[:, sl], ps[:], pe1q_sb[:, sl])

                def k_proj(jb):
                    sl = bass.ts(jb, 512)
                    ps = psP.tile([128, 512], F32, name="psp")[0:QK, :]
                    for kt in range(2):
                        nc.tensor.matmul(ps[:], wqk_sb[:, 128 + kt * QK:
                                                       128 + (kt + 1) * QK],
                                         xb_sb[:, jb * 1024 + kt * 512:
                                               jb * 1024 + (kt + 1) * 512],
                                         start=(kt == 0), stop=(kt == 1))
                    nc.vector.tensor_add(k_sb[:, sl], ps[:], pe1_sb[:, sl])

                def v_pair(jt0):
                    """v^T for key tiles jt0, jt0+1 into one [128,512] psum;
                    the v bias rides a K=1 rank-1 matmul; the den reciprocal
                    folds in during the single DVE evacuation per tile."""
                    ps = psP.tile([128, 512], F32, name="psp")
                    for r in range(2):
                        jt = jt0 + r
                        jb, jr = jt // 4, jt % 4
                        half = ps[:, r * 256:(r + 1) * 256]
                        for kt in range(2):
                            nc.tensor.matmul(
                                half,
                                xb_sb[:, jb * 1024 + kt * 512 + jr * 128:
                                      jb * 1024 + kt * 512 + (jr + 1) * 128],
                                wvt[:, bass.ts(kt, 256)],
                                start=(kt == 0), stop=False,
                                skip_group_check=True)
                        nc.tensor.matmul(half, ones_sb[:], bvb[0:1, :],
                                         start=False, stop=True,
                                         skip_group_check=True)
                    vts_sb = vts_cell[0]
                    for r in range(2):
                        jt = jt0 + r
                        nc.vector.tensor_scalar_mul(
                            vts_sb[:, bass.ts(jt, 256)],
                            ps[:, r * 256:(r + 1) * 256],
                            rden_sb[:, jt:jt + 1])

                def s_half(jt, h):
                    """S[j, i-half] = (k^T q)/8 then exp; two [128,1024]
                    PSUM buffers keep next-half matmuls under this half's
                    exp."""
                    ps = psS.tile([128, 1024], F32, name="pss")
                    for i2 in range(2):
                        ib = 2 * h + i2
                        nc.tensor.matmul(ps[:, bass.ts(i2, 512)],
                                         k_sb[:, bass.ts(jt, 128)],
                                         q_sb[:, bass.ts(ib, 512)],
                                         start=True, stop=True)
                    nc.scalar.activation(
                        attn_sb[:, jt * IS + h * 1024:jt * IS + (h + 1) * 1024],
                        ps[:], AF.Exp, scale=0.125,
                        accum_out=den_h[:, 2 * jt + h:2 * jt + h + 1])

                def den_chunk(ci):
                    lo, hi = CHUNKS[ci]
                    w = hi - lo
                    nc.vector.tensor_add(den_sb[:, lo:hi],
                                         den_h[:, 2 * lo:2 * hi:2],
                                         den_h[:, 2 * lo + 1:2 * hi:2])
                    den_in = dram.tile([128, w], F32, name=f"den_in{ci}")
                    den_out = dram.tile([128, w], F32, name=f"den_out{ci}")
                    nc.sync.dma_start(den_in[:], den_sb[:, lo:hi])
                    nc.gpsimd.collective_compute(
                        "AllReduce", OP.add,
                        replica_groups=REPLICA_GROUPS,
                        ins=[den_in.opt()], outs=[den_out.opt()],
                    )
                    nc.sync.dma_start(dsum_sb[:, lo:hi], den_out[:])
                    nc.vector.reciprocal(rden_sb[:, lo:hi], dsum_sb[:, lo:hi])

                def av_group(ci, mt, ib):
                    lo, hi = CHUNKS[ci]
                    ps = psO.tile([128, 512], F32, name="psav")
                    vts_sb = vts_cell[0]
                    for jt in range(lo, hi):
                        nc.tensor.matmul(
                            ps[:],
                            vts_sb[:, jt * 256 + mt * 128:
                                   jt * 256 + (mt + 1) * 128],
                            attn_sb[:, jt * IS + ib * 512:
                                    jt * IS + (ib + 1) * 512],
                            start=(jt == lo), stop=(jt == hi - 1),
                            skip_group_check=True)
                    sl = bass.ts(ib, 512)
                    if ci == 0:
                        nc.vector.tensor_copy(oacc[mt][:, sl], ps[:])
                    else:
                        nc.vector.tensor_add(oacc[mt][:, sl], ps[:],
                                             oacc[mt][:, sl])

                # ---- emission schedule ----
                for ib in range(NIB):
                    q_proj(ib)
                k_proj(0)
                k_proj(1)

                chunk_of_end = {hi - 1: ci for ci, (lo, hi) in enumerate(CHUNKS)}
                # av groups: chunk ci's groups start 3 key tiles after its
                # allreduce is issued, one group per key-tile slot.
                av_sched = {}
                for ci, (lo, hi) in enumerate(CHUNKS):
                    for g, (mt, ib) in enumerate(
                            (m, i) for i in range(NIB) for m in range(2)):
                        av_sched.setdefault(hi + 2 + g, []).append((ci, mt, ib))

                post = []
                for jt in range(NJT):
                    s_half(jt, 0)
                    if jt < 6:
                        k_proj(jt + 2)
                    s_half(jt, 1)
                    if jt == 1:
                        pe1qpool.release()
                        xqpool.release()
                    if jt == 7:
                        pe1pool.release()
                    if jt == 0:
                        vtspool = tc.alloc_tile_pool(name="vts", bufs=1,
                                                     side="right")
                        vts_cell[0] = vtspool.tile([128, NJT * 256], BF16,
                                                   name="vts_sb")
                    if jt == 2:
                        opool = tc.alloc_tile_pool(name="oacc", bufs=1,
                                                   side="right")
                        for m in range(2):
                            oacc[m] = opool.tile([128, IS], F32,
                                                 name=f"oacc{m}")
                    ci = chunk_of_end.get(jt)
                    if ci is not None:
                        den_chunk(ci)
                    if jt - 1 in chunk_of_end:
                        ci2 = chunk_of_end[jt - 1]
                        for j2 in range(*CHUNKS[ci2], 2):
                            v_pair(j2)
                    for item in av_sched.pop(jt, []):
                        if jt < NJT - 1:
                            av_group(*item)
                        else:
                            post.append(item)
                # final chunk's v folds
                for j2 in range(CHUNKS[-1][0], CHUNKS[-1][1], 2):
                    v_pair(j2)
                for jts in sorted(av_sched):
                    post.extend(av_sched[jts])
                xbpool.release()

                # remaining attn@v groups (ib-major so the MLP can chase)
                post.sort(key=lambda t: (t[0], t[2], t[1]))
                for item in post:
                    av_group(*item)

                # ---- MLP tail ----
                # h = mish(W1 out + b1) via the exact identity
                #   mish(x) = x * t1 * r,  t1 = (e+2)*e,  r = 1/(t1+2)
                # (one ScalarE Exp - same table as the attention stream, zero
                # table switches - plus a 4-op DVE chain; DVE reciprocal needs
                # no LUT).  y = W2 h + b2 + x.
                with (
                    tc.tile_pool(name="outb", bufs=1) as outpool,
                    tc.tile_pool(name="xf", bufs=1) as xfpool,
                    tc.tile_pool(name="h", bufs=2) as hpool,
                    tc.tile_pool(name="mt", bufs=2) as mpool,
                    tc.tile_pool(name="y", bufs=1) as ypool,
                ):
                    out_sb = [outpool.tile([128, IS], BF16, name=f"out{m}")
                              for m in range(2)]
                    for mt in range(2):
                        for ib in range(NIB):
                            # ScalarE Identity copy: DVE is busy with the
                            # final accumulator adds at this point.
                            nc.scalar.activation(
                                out_sb[mt][:, bass.ts(ib, 512)],
                                oacc[mt][:, bass.ts(ib, 512)], AF.Identity)
                    xf_sb = xfpool.tile([128, 2 * IS], BF16)
                    nc.gpsimd.dma_start(xf_sb[:], xf_d[:])

                    hs = {}
                    for ib in range(NIB):
                        sl = bass.ts(ib, 512)
                        for mt in range(2):
                            ps = psO.tile([128, 512], F32, name="psav")
                            for kt in range(2):
                                nc.tensor.matmul(
                                    ps[:],
                                    w1t[:, kt * 256 + mt * 128:
                                        kt * 256 + (mt + 1) * 128],
                                    out_sb[kt][:, sl],
                                    start=(kt == 0), stop=(kt == 1),
                                    skip_group_check=True)
                            e_t = mpool.tile([128, 512], BF16, name="mish_e")
                            t1_t = mpool.tile([128, 512], BF16, name="mish_t1")
                            r_t = mpool.tile([128, 512], BF16, name="mish_r")
                            m_t = mpool.tile([128, 512], BF16, name="mish_m")
                            h_t = hpool.tile([128, 512], BF16, name="mish_h")
                            nc.scalar.activation(e_t[:], ps[:], AF.Exp,
                                                 bias=b1c[:, mt:mt + 1])
                            nc.vector.scalar_tensor_tensor(
                                t1_t[:], e_t[:], 2.0, e_t[:],
                                op0=OP.add, op1=OP.mult)
                            nc.vector.tensor_scalar_add(r_t[:], t1_t[:], 2.0)
                            with nc.allow_low_precision(
                                    reason="mish: bf16 reciprocal, error "
                                           "averages out across 256 ch"):
                                nc.vector.reciprocal(r_t[:], r_t[:])
                            nc.vector.tensor_mul(m_t[:], t1_t[:], r_t[:])
                            nc.vector.scalar_tensor_tensor(
                                h_t[:], ps[:], b1c[:, mt:mt + 1],
                                m_t[:], op0=OP.add, op1=OP.mult)
                            hs[mt, ib] = h_t
                        for mt in range(2):
                            ps = psO.tile([128, 512], F32, name="psav")
                            for kt in range(2):
                                nc.tensor.matmul(
                                    ps[:],
                                    w2t[:, kt * 256 + mt * 128:
                                        kt * 256 + (mt + 1) * 128],
                                    hs[kt, ib][:],
                                    start=(kt == 0), stop=(kt == 1),
                                    skip_group_check=True)
                            y_sb = ypool.tile([128, 512], F32)
                            nc.vector.scalar_tensor_tensor(
                                y_sb[:], ps[:], b2c[:, mt:mt + 1],
                                xf_sb[:, mt * IS + ib * 512:
                                      mt * IS + (ib + 1) * 512],
                                op0=OP.add, op1=OP.add)
                            nc.sync.dma_start(
                                y_d[mt * 128:(mt + 1) * 128, sl], y_sb[:])
                opool.release()
                vtspool.release()
    nc.finalize()
    return nc


def _to_lhsT_sb(w):
    """[256, M] fp32 -> SBUF layout [128, 2*M] bf16: col block kt holds rows
    kt*128..kt*128+127 of w."""
    k, m = w.shape
    assert k == 256
    return np.ascontiguousarray(
        w.reshape(2, 128, m).transpose(1, 0, 2).reshape(128, 2 * m).astype(bf16))


def _bf(a):
    return np.ascontiguousarray(np.asarray(a, dtype=np.float32).astype(bf16))


def _halves(a):
    """[256, X] -> [128, 2*X] with the two 128-row halves side by side."""
    return np.ascontiguousarray(np.concatenate([a[:128], a[128:]], axis=1))


def _blocks(a, blk=512):
    """[256, X] -> [128, 2*X] with 512-col blocks: block jb holds
    [rows 0-127 | rows 128-255] of cols jb*512..+512."""
    k, x = a.shape
    nb = x // blk
    out = np.empty((128, 2 * x), a.dtype)
    for jb in range(nb):
        out[:, jb * 2 * blk:jb * 2 * blk + blk] = a[:128, jb * blk:(jb + 1) * blk]
        out[:, jb * 2 * blk + blk:(jb + 1) * 2 * blk] = a[128:, jb * blk:(jb + 1) * blk]
    return np.ascontiguousarray(out)


def make_in_maps(x, WQ, bQ, WK, bK, WV, bV, PE, W1, b1, W2, b2, n_cores=N_CORES):
    x = np.asarray(x, dtype=np.float32)
    xf3 = np.ascontiguousarray(x.reshape(B, C, N))
    pef = np.asarray(PE, dtype=np.float32).reshape(QK, N)
    pe1 = _bf(pef + np.asarray(bK, np.float32)[:, None])
    pe1q_full = _bf(pef + np.asarray(bQ, np.float32)[:, None])

    wq = _to_lhsT_sb(np.asarray(WQ, np.float32).T)   # [128, 128]
    wk = _to_lhsT_sb(np.asarray(WK, np.float32).T)
    wmlp = np.concatenate([
        _to_lhsT_sb(np.asarray(WV, np.float32).T),
        _to_lhsT_sb(np.asarray(W1, np.float32).T),
        _to_lhsT_sb(np.asarray(W2, np.float32).T),
        np.broadcast_to(_bf(np.asarray(bV)[None, :]), (128, 256)),
    ], axis=1)
    bcols = np.concatenate([
        np.asarray(b1, np.float32).reshape(2, 128).T,
        np.asarray(b2, np.float32).reshape(2, 128).T,
    ], axis=1)

    shared = {
        "pe1": pe1,
        "wqk": np.ascontiguousarray(np.concatenate([wq, wk], axis=1)),
        "wmlp": np.ascontiguousarray(wmlp),
        "bcols": np.ascontiguousarray(bcols),
    }
    in_maps = []
    for core in range(n_cores):
        s, h = core // 2, core % 2
        isl = slice(h * IS, (h + 1) * IS)
        xb = _bf(xf3[s])
        m = dict(shared)
        m["xb"] = _blocks(xb)
        # xq is ib-interleaved: [:, ib*1024+kt*512 : ...] = channel-half kt,
        # query block ib - so the first DMA chunk covers ib 0-1 completely.
        xqs = xb[:, isl]
        m["xq"] = np.ascontiguousarray(np.concatenate(
            [np.concatenate([xqs[:128, ib * 512:(ib + 1) * 512],
                             xqs[128:, ib * 512:(ib + 1) * 512]], axis=1)
             for ib in range(NIB)], axis=1))
        m["xf"] = _halves(_bf(xf3[s][:, isl]))
        m["pe1q"] = np.ascontiguousarray(pe1q_full[:, isl])
        in_maps.append(m)
    return in_maps


def assemble_output(results, n_cores=N_CORES):
    y = np.empty((B, C, N), dtype=np.float32)
    for s in range(B):
        y[s][:, :IS] = results[2 * s]["y"]
        y[s][:, IS:] = results[2 * s + 1]["y"]
    return y.reshape(B, C, H, W)


_PROG = None


def kernel(**inputs) -> np.ndarray:
    global _PROG
    from concourse.bass_utils import run_bass_kernel_spmd
    if _PROG is None:
        _PROG = build_program(N_CORES)
    in_maps = make_in_maps(**inputs)
    res = run_bass_kernel_spmd(_PROG, in_maps, core_ids=list(range(N_CORES)))
    return assemble_output(res.results)


# revision 18
# speedup vs baseline: 1.0961x; 1.0746x over previous
"""AttentionHead kernel for 8 Trainium2 NeuronCores.

Problem (per sample, B=4): x:[256,64,64] -> q/k/v 1x1-conv projections
(+positional encoding on q,k), S = q^T k / 8, softmax over the QUERY axis,
out = attn @ v, then 1x1-conv MLP with Mish + residual.

Sharding: 2 cores per sample, split over the query axis i (2048 queries each).
Softmax normalizes over i, so the per-key denominator den[j] = sum_i exp(S[i,j])
needs one tiny AllReduce per core pair (done in 4 chunks, latency hidden);
den folds into v (v/den), everything else is local, output halves disjoint.

v2 restructure vs the phase-separated baseline: the ScalarE exp stream
(~72us for 32x [128,2048] Exp activations) is the pacing resource, so the
attn@v matmuls are interleaved INTO the exp stream instead of running as a
separate phase afterwards.  PSUM: one [128,2048] S tile (4 banks) + three
[128,512] slots (3 banks) shared by attn@v chunk-accumulation and all
projection/MLP matmuls.  attn@v accumulates per 8-key-tile chunk in PSUM and
is folded into an SBUF fp32 accumulator by DVE adds, so chunk c's matmuls run
while chunk c+1's S tiles are still being exp'd.  This keeps the PE busy
end-to-end (no >3.4us idle -> no HAM re-throttle to 1.2GHz) and removes the
55us serialized attn@v phase.

Layout trick (unchanged): compute S transposed, S[j,i] = (k^T q)[j,i], keys j
on partitions.  exp runs PSUM->SBUF with a per-partition accumulate (the
denominator for free), and exp(S)[j,i] is directly the correct operand layout
for both out[c,i] = sum_j v[c,j]*attnT[j,i] and the MLP - zero transposes.
All matmul operands bf16 (fp32 PSUM accumulation).

MLP uses the hardware Mish LUT: h = Mish(psum + b1) in one activation.
Biases: q/k biases fold into the positional-encoding tensors on the host; the
v bias is added during the PSUM->SBUF move; b2 rides the final residual add.
"""

import numpy as np
import ml_dtypes

import concourse.bass as bass
import concourse.bacc as bacc
import concourse.mybir as mybir
import concourse.tile as tile

BF16 = mybir.dt.bfloat16
F32 = mybir.dt.float32
AF = mybir.ActivationFunctionType
OP = mybir.AluOpType
bf16 = ml_dtypes.bfloat16

B, C, H, W = 4, 256, 64, 64
N = H * W            # 4096 pixels
QK = 64
IS = N // 2          # 2048 queries per core
NJT = N // 128       # 32 key tiles
NIB = IS // 512      # 4 i-blocks
NCH = 4              # den allreduce chunks
JCH = NJT // NCH     # 8 key tiles per chunk
N_CORES = 8
REPLICA_GROUPS = [[0, 1], [2, 3], [4, 5], [6, 7]]


def build_program(n_cores: int = N_CORES, enable_asserts: bool = False) -> bass.Bass:
    nc = bacc.Bacc(
        "TRN2",
        target_bir_lowering=False,
        debug=False,
        enable_asserts=enable_asserts,
        num_devices=n_cores,
    )

    # Per-core inputs (data differs by core; program is identical).
    # xq: q-projection input, ib-interleaved: [:, ib*1024+kt*512+c] is channel
    #   half kt, query block ib.
    # xb: k/v-projection input, column-block-interleaved: [:, jb*1024+kt*512+c]
    #   is channel half kt, pixel columns jb*512+c.
    xq_d = nc.dram_tensor("xq", [128, 2 * IS], BF16, kind="ExternalInput").ap()
    xb_d = nc.dram_tensor("xb", [128, 2 * N], BF16, kind="ExternalInput").ap()
    xf_d = nc.dram_tensor("xf", [128, 2 * IS], BF16, kind="ExternalInput").ap()
    pe1q_d = nc.dram_tensor("pe1q", [QK, IS], BF16, kind="ExternalInput").ap()
    # Shared weights (same on all cores).
    pe1_d = nc.dram_tensor("pe1", [QK, N], BF16, kind="ExternalInput").ap()
    wqk_d = nc.dram_tensor("wqk", [128, 256], BF16, kind="ExternalInput").ap()
    # wmlp = wvt | w1t | w2t | bvb
    wmlp_d = nc.dram_tensor("wmlp", [128, 1792], BF16, kind="ExternalInput").ap()
    bcols_d = nc.dram_tensor("bcols", [128, 4], F32, kind="ExternalInput").ap()

    y_d = nc.dram_tensor("y", [C, IS], F32, kind="ExternalOutput").ap()

    # den-allreduce chunks (key-tile ranges).  Front chunks are small so the
    # first attn@v groups unlock early and fill the PE during the exp stream.
    CHUNKS = [(0, 2), (2, 4), (4, 8), (8, 16), (16, 24), (24, 32)]

    with tile.TileContext(nc) as tc:
        with (
            tc.tile_pool(name="const", bufs=1) as cpool,
            tc.tile_pool(name="qk", bufs=1) as qkpool,
            tc.tile_pool(name="den", bufs=1) as denpool,
            tc.tile_pool(name="dram", bufs=1, space="DRAM") as dram,
            tc.tile_pool(name="psS", bufs=2, space="PSUM") as psS,
            tc.tile_pool(name="psO", bufs=2, space="PSUM") as psO,
            tc.tile_pool(name="psP", bufs=2, space="PSUM") as psP,
        ):
            wqk_sb = cpool.tile([128, 256], BF16)
            nc.sync.dma_start(wqk_sb[:], wqk_d[:])

            q_sb = qkpool.tile([QK, IS], BF16)     # q, d on partitions
            k_sb = qkpool.tile([QK, N], BF16)      # k, d on partitions
            den_sb = denpool.tile([128, NJT], F32)
            dsum_sb = denpool.tile([128, NJT], F32)
            rden_sb = denpool.tile([128, NJT], F32)
            den_h = denpool.tile([128, 2 * NJT], F32)   # per-half accums
            ones_sb = cpool.tile([1, 128], BF16)
            nc.vector.memset(ones_sb[:], 1.0)
            dummy_in = None

            with tc.tile_pool(name="attn", bufs=1) as apool:
                attn_sb = apool.tile([128, NJT * IS], BF16)   # 16 MiB
                xbpool = tc.alloc_tile_pool(name="xb", bufs=1)
                pe1pool = tc.alloc_tile_pool(name="pe1", bufs=1)
                xqpool = tc.alloc_tile_pool(name="xq", bufs=1)
                pe1qpool = tc.alloc_tile_pool(name="pe1q", bufs=1)
                xb_sb = xbpool.tile([128, 2 * N], BF16)
                xq_sb = xqpool.tile([128, 2 * IS], BF16)
                pe1q_sb = pe1qpool.tile([QK, IS], BF16)
                pe1_sb = pe1pool.tile([QK, N], BF16)
                nc.sync.dma_start(xq_sb[:, bass.ts(0, IS)],
                                  xq_d[:, bass.ts(0, IS)])
                nc.sync.dma_start(pe1q_sb[:], pe1q_d[:])
                nc.sync.dma_start(xb_sb[:, bass.ts(0, N // 2)],
                                  xb_d[:, bass.ts(0, N // 2)])
                nc.sync.dma_start(pe1_sb[:, bass.ts(0, N // 2)],
                                  pe1_d[:, bass.ts(0, N // 2)])
                nc.sync.dma_start(xb_sb[:, bass.ts(1, N // 2)],
                                  xb_d[:, bass.ts(1, N // 2)])
                nc.sync.dma_start(pe1_sb[:, bass.ts(1, N // 2)],
                                  pe1_d[:, bass.ts(1, N // 2)])
                nc.sync.dma_start(xq_sb[:, bass.ts(1, IS)],
                                  xq_d[:, bass.ts(1, IS)])
                for ch in range(2, 4):
                    nc.sync.dma_start(xb_sb[:, bass.ts(ch, N // 2)],
                                      xb_d[:, bass.ts(ch, N // 2)])
                # CC warm-up: the first collective pays a multi-us mesh
                # barrier + setup; run a 1-element dummy AllReduce now so the
                # real den reductions are cheap.
                warm_in = dram.tile([1, 4], F32, name="warm_in")
                warm_out = dram.tile([1, 4], F32, name="warm_out")
                nc.gpsimd.dma_start(warm_in[:], bcols_d[0:1, 0:4])
                nc.gpsimd.collective_compute(
                    "AllReduce", OP.add,
                    replica_groups=REPLICA_GROUPS,
                    ins=[warm_in.opt()], outs=[warm_out.opt()],
                )
                wmlp_sb = cpool.tile([128, 1792], BF16)
                bcols_sb = cpool.tile([128, 4], F32)
                nc.gpsimd.dma_start(wmlp_sb[:], wmlp_d[:])
                nc.gpsimd.dma_start(bcols_sb[:], bcols_d[:])
                wvt = wmlp_sb[:, 0:512]
                w1t = wmlp_sb[:, 512:1024]
                w2t = wmlp_sb[:, 1024:1536]
                bvb = wmlp_sb[:, 1536:1792]
                b1c = bcols_sb[:, 0:2]
                b2c = bcols_sb[:, 2:4]

                # lazily allocated, on the right stack
                vts_cell = [None]     # v^T/den, [128, NJT*256] bf16
                oacc = [None, None]   # fp32 attn-out accumulators

                def q_proj(ib):
                    sl = bass.ts(ib, 512)
                    ps = psP.tile([128, 512], F32, name="psp")[0:QK, :]
                    for kt in range(2):
                        nc.tensor.matmul(ps[:], wqk_sb[:, bass.ts(kt, QK)],
                                         xq_sb[:, ib * 1024 + kt * 512:
                                               ib * 1024 + (kt + 1) * 512],
                                         start=(kt == 0), stop=(kt == 1))
                    nc.vector.tensor_add(q_sb[:, sl], ps[:], pe1q_sb[:, sl])

                def k_proj(jb):
                    sl = bass.ts(jb, 512)
                    ps = psP.tile([128, 512], F32, name="psp")[0:QK, :]
                    for kt in range(2):
                        nc.tensor.matmul(ps[:], wqk_sb[:, 128 + kt * QK:
                                                       128 + (kt + 1) * QK],
                                         xb_sb[:, jb * 1024 + kt * 512:
                                               jb * 1024 + (kt + 1) * 512],
                                         start=(kt == 0), stop=(kt == 1))
                    nc.vector.tensor_add(k_sb[:, sl], ps[:], pe1_sb[:, sl])

                def v_pair(jt0):
                    """v^T for key tiles jt0, jt0+1 into one [128,512] psum;
                    the v bias rides a K=1 rank-1 matmul; the den reciprocal
                    folds in during the single DVE evacuation per tile."""
                    ps = psP.tile([128, 512], F32, name="psp")
                    for r in range(2):
                        jt = jt0 + r
                        jb, jr = jt // 4, jt % 4
                        half = ps[:, r * 256:(r + 1) * 256]
                        for kt in range(2):
                            nc.tensor.matmul(
                                half,
                                xb_sb[:, jb * 1024 + kt * 512 + jr * 128:
                                      jb * 1024 + kt * 512 + (jr + 1) * 128],
                                wvt[:, bass.ts(kt, 256)],
                                start=(kt == 0), stop=False,
                                skip_group_check=True)
                        nc.tensor.matmul(half, ones_sb[:], bvb[0:1, :],
                                         start=False, stop=True,
                                         skip_group_check=True)
                    vts_sb = vts_cell[0]
                    for r in range(2):
                        jt = jt0 + r
                        nc.vector.tensor_scalar_mul(
                            vts_sb[:, bass.ts(jt, 256)],
                            ps[:, r * 256:(r + 1) * 256],
                            rden_sb[:, jt:jt + 1])

                def s_half(jt, h):
                    """S[j, i-half] = (k^T q)/8 then exp; two [128,1024]
                    PSUM buffers keep next-half matmuls under this half's
                    exp."""
                    ps = psS.tile([128, 1024], F32, name="pss")
                    for i2 in range(2):
                        ib = 2 * h + i2
                        nc.tensor.matmul(ps[:, bass.ts(i2, 512)],
                                         k_sb[:, bass.ts(jt, 128)],
                                         q_sb[:, bass.ts(ib, 512)],
                                         start=True, stop=True)
                    nc.scalar.activation(
                        attn_sb[:, jt * IS + h * 1024:jt * IS + (h + 1) * 1024],
                        ps[:], AF.Exp, scale=0.125,
                        accum_out=den_h[:, 2 * jt + h:2 * jt + h + 1])

                def den_chunk(ci):
                    lo, hi = CHUNKS[ci]
                    w = hi - lo
                    nc.vector.tensor_add(den_sb[:, lo:hi],
                                         den_h[:, 2 * lo:2 * hi:2],
                                         den_h[:, 2 * lo + 1:2 * hi:2])
                    den_in = dram.tile([128, w], F32, name=f"den_in{ci}")
                    den_out = dram.tile([128, w], F32, name=f"den_out{ci}")
                    nc.gpsimd.dma_start(den_in[:], den_sb[:, lo:hi])
                    nc.gpsimd.collective_compute(
                        "AllReduce", OP.add,
                        replica_groups=REPLICA_GROUPS,
                        ins=[den_in.opt()], outs=[den_out.opt()],
                    )
                    nc.gpsimd.dma_start(dsum_sb[:, lo:hi], den_out[:])
                    nc.vector.reciprocal(rden_sb[:, lo:hi], dsum_sb[:, lo:hi])

                def av_group(ci, mt, ib):
                    lo, hi = CHUNKS[ci]
                    ps = psO.tile([128, 512], F32, name="psav")
                    vts_sb = vts_cell[0]
                    for jt in range(lo, hi):
                        nc.tensor.matmul(
                            ps[:],
                            vts_sb[:, jt * 256 + mt * 128:
                                   jt * 256 + (mt + 1) * 128],
                            attn_sb[:, jt * IS + ib * 512:
                                    jt * IS + (ib + 1) * 512],
                            start=(jt == lo), stop=(jt == hi - 1),
                            skip_group_check=True)
                    sl = bass.ts(ib, 512)
                    if ci == 0:
                        nc.vector.tensor_copy(oacc[mt][:, sl], ps[:])
                    else:
                        nc.vector.tensor_add(oacc[mt][:, sl], ps[:],
                                             oacc[mt][:, sl])

                # ---- emission schedule ----
                for ib in range(NIB):
                    q_proj(ib)
                k_proj(0)
                k_proj(1)

                chunk_of_end = {hi - 1: ci for ci, (lo, hi) in enumerate(CHUNKS)}
                # av groups: chunk ci's groups start 3 key tiles after its
                # allreduce is issued, one group per key-tile slot.
                av_sched = {}
                for ci, (lo, hi) in enumerate(CHUNKS):
                    for g, (mt, ib) in enumerate(
                            (m, i) for i in range(NIB) for m in range(2)):
                        av_sched.setdefault(hi + 2 + g, []).append((ci, mt, ib))

                post = []
                for jt in range(NJT):
                    s_half(jt, 0)
                    s_half(jt, 1)
                    if jt == 1:
                        pe1qpool.release()
                        xqpool.release()
                    if jt == 7:
                        pe1pool.release()
                    if jt == 0:
                        vtspool = tc.alloc_tile_pool(name="vts", bufs=1,
                                                     side="right")
                        vts_cell[0] = vtspool.tile([128, NJT * 256], BF16,
                                                   name="vts_sb")
                    if jt == 2:
                        opool = tc.alloc_tile_pool(name="oacc", bufs=1,
                                                   side="right")
                        for m in range(2):
                            oacc[m] = opool.tile([128, IS], F32,
                                                 name=f"oacc{m}")
                    ci = chunk_of_end.get(jt)
                    if ci is not None:
                        den_chunk(ci)
                    if jt - 1 in chunk_of_end:
                        ci2 = chunk_of_end[jt - 1]
                        for j2 in range(*CHUNKS[ci2], 2):
                            v_pair(j2)
                    if jt < 6:
                        k_proj(jt + 2)
                    for item in av_sched.pop(jt, []):
                        if jt < NJT - 1:
                            av_group(*item)
                        else:
                            post.append(item)
                # final chunk's v folds
                for j2 in range(CHUNKS[-1][0], CHUNKS[-1][1], 2):
                    v_pair(j2)
                for jts in sorted(av_sched):
                    post.extend(av_sched[jts])
                xbpool.release()

                # remaining attn@v groups (ib-major so the MLP can chase)
                post.sort(key=lambda t: (t[0], t[2], t[1]))
                for item in post:
                    av_group(*item)

                # ---- MLP tail ----
                # h = mish(W1 out + b1) via the exact identity
                #   mish(x) = x * t1 * r,  t1 = (e+2)*e,  r = 1/(t1+2)
                # (one ScalarE Exp - same table as the attention stream, zero
                # table switches - plus a 4-op DVE chain; DVE reciprocal needs
                # no LUT).  y = W2 h + b2 + x.
                with (
                    tc.tile_pool(name="outb", bufs=1) as outpool,
                    tc.tile_pool(name="xf", bufs=1) as xfpool,
                    tc.tile_pool(name="h", bufs=2) as hpool,
                    tc.tile_pool(name="mt", bufs=2) as mpool,
                    tc.tile_pool(name="y", bufs=1) as ypool,
                ):
                    out_sb = [outpool.tile([128, IS], BF16, name=f"out{m}")
                              for m in range(2)]
                    for mt in range(2):
                        for ib in range(NIB):
                            # ScalarE Identity copy: DVE is busy with the
                            # final accumulator adds at this point.
                            nc.scalar.activation(
                                out_sb[mt][:, bass.ts(ib, 512)],
                                oacc[mt][:, bass.ts(ib, 512)], AF.Identity)
                    xf_sb = xfpool.tile([128, 2 * IS], BF16)
                    nc.gpsimd.dma_start(xf_sb[:], xf_d[:])

                    hs = {}
                    for ib in range(NIB):
                        sl = bass.ts(ib, 512)
                        for mt in range(2):
                            ps = psO.tile([128, 512], F32, name="psav")
                            for kt in range(2):
                                nc.tensor.matmul(
                                    ps[:],
                                    w1t[:, kt * 256 + mt * 128:
                                        kt * 256 + (mt + 1) * 128],
                                    out_sb[kt][:, sl],
                                    start=(kt == 0), stop=(kt == 1),
                                    skip_group_check=True)
                            e_t = mpool.tile([128, 512], BF16, name="mish_e")
                            t1_t = mpool.tile([128, 512], BF16, name="mish_t1")
                            r_t = mpool.tile([128, 512], BF16, name="mish_r")
                            m_t = mpool.tile([128, 512], BF16, name="mish_m")
                            h_t = hpool.tile([128, 512], BF16, name="mish_h")
                            nc.scalar.activation(e_t[:], ps[:], AF.Exp,
                                                 bias=b1c[:, mt:mt + 1])
                            nc.vector.scalar_tensor_tensor(
                                t1_t[:], e_t[:], 2.0, e_t[:],
                                op0=OP.add, op1=OP.mult)
                            nc.vector.tensor_scalar_add(r_t[:], t1_t[:], 2.0)
                            with nc.allow_low_precision(
                                    reason="mish: bf16 reciprocal, error "
                                           "averages out across 256 ch"):
                                nc.vector.reciprocal(r_t[:], r_t[:])
                            nc.vector.tensor_mul(m_t[:], t1_t[:], r_t[:])
                            nc.vector.scalar_tensor_tensor(
                                h_t[:], ps[:], b1c[:, mt:mt + 1],
                                m_t[:], op0=OP.add, op1=OP.mult)
                            hs[mt, ib] = h_t
                        for mt in range(2):
                            ps = psO.tile([128, 512], F32, name="psav")
                            for kt in range(2):
                                nc.tensor.matmul(
                                    ps[:],
                                    w2t[:, kt * 256 + mt * 128:
                                        kt * 256 + (mt + 1) * 128],
                                    hs[kt, ib][:],
                                    start=(kt == 0), stop=(kt == 1),
                                    skip_group_check=True)
                            y_sb = ypool.tile([128, 512], F32)
                            nc.vector.scalar_tensor_tensor(
                                y_sb[:], ps[:], b2c[:, mt:mt + 1],
                                xf_sb[:, mt * IS + ib * 512:
                                      mt * IS + (ib + 1) * 512],
                                op0=OP.add, op1=OP.add)
                            nc.sync.dma_start(
                                y_d[mt * 128:(mt + 1) * 128, sl], y_sb[:])
                opool.release()
                vtspool.release()
    nc.finalize()
    return nc


def _to_lhsT_sb(w):
    """[256, M] fp32 -> SBUF layout [128, 2*M] bf16: col block kt holds rows
    kt*128..kt*128+127 of w."""
    k, m = w.shape
    assert k == 256
    return np.ascontiguousarray(
        w.reshape(2, 128, m).transpose(1, 0, 2).reshape(128, 2 * m).astype(bf16))


def _bf(a):
    return np.ascontiguousarray(np.asarray(a, dtype=np.float32).astype(bf16))


def _halves(a):
    """[256, X] -> [128, 2*X] with the two 128-row halves side by side."""
    return np.ascontiguousarray(np.concatenate([a[:128], a[128:]], axis=1))


def _blocks(a, blk=512):
    """[256, X] -> [128, 2*X] with 512-col blocks: block jb holds
    [rows 0-127 | rows 128-255] of cols jb*512..+512."""
    k, x = a.shape
    nb = x // blk
    out = np.empty((128, 2 * x), a.dtype)
    for jb in range(nb):
        out[:, jb * 2 * blk:jb * 2 * blk + blk] = a[:128, jb * blk:(jb + 1) * blk]
        out[:, jb * 2 * blk + blk:(jb + 1) * 2 * blk] = a[128:, jb * blk:(jb + 1) * blk]
    return np.ascontiguousarray(out)


def make_in_maps(x, WQ, bQ, WK, bK, WV, bV, PE, W1, b1, W2, b2, n_cores=N_CORES):
    x = np.asarray(x, dtype=np.float32)
    xf3 = np.ascontiguousarray(x.reshape(B, C, N))
    pef = np.asarray(PE, dtype=np.float32).reshape(QK, N)
    pe1 = _bf(pef + np.asarray(bK, np.float32)[:, None])
    pe1q_full = _bf(pef + np.asarray(bQ, np.float32)[:, None])

    wq = _to_lhsT_sb(np.asarray(WQ, np.float32).T)   # [128, 128]
    wk = _to_lhsT_sb(np.asarray(WK, np.float32).T)
    wmlp = np.concatenate([
        _to_lhsT_sb(np.asarray(WV, np.float32).T),
        _to_lhsT_sb(np.asarray(W1, np.float32).T),
        _to_lhsT_sb(np.asarray(W2, np.float32).T),
        np.broadcast_to(_bf(np.asarray(bV)[None, :]), (128, 256)),
    ], axis=1)
    bcols = np.concatenate([
        np.asarray(b1, np.float32).reshape(2, 128).T,
        np.asarray(b2, np.float32).reshape(2, 128).T,
    ], axis=1)

    shared = {
        "pe1": pe1,
        "wqk": np.ascontiguousarray(np.concatenate([wq, wk], axis=1)),
        "wmlp": np.ascontiguousarray(wmlp),
        "bcols": np.ascontiguousarray(bcols),
    }
    in_maps = []
    for core in range(n_cores):
        s, h = core // 2, core % 2
        isl = slice(h * IS, (h + 1) * IS)
        xb = _bf(xf3[s])
        m = dict(shared)
        m["xb"] = _blocks(xb)
        # xq is ib-interleaved: [:, ib*1024+kt*512 : ...] = channel-half kt,
        # query block ib - so the first DMA chunk covers ib 0-1 completely.
        xqs = xb[:, isl]
        m["xq"] = np.ascontiguousarray(np.concatenate(
            [np.concatenate([xqs[:128, ib * 512:(ib + 1) * 512],
                             xqs[128:, ib * 512:(ib + 1) * 512]], axis=1)
             for ib in range(NIB)], axis=1))
        m["xf"] = _halves(_bf(xf3[s][:, isl]))
        m["pe1q"] = np.ascontiguousarray(pe1q_full[:, isl])
        in_maps.append(m)
    return in_maps


def assemble_output(results, n_cores=N_CORES):
    y = np.empty((B, C, N), dtype=np.float32)
    for s in range(B):
        y[s][:, :IS] = results[2 * s]["y"]
        y[s][:, IS:] = results[2 * s + 1]["y"]
    return y.reshape(B, C, H, W)


_PROG = None


def kernel(**inputs) -> np.ndarray:
    global _PROG
    from concourse.bass_utils import run_bass_kernel_spmd
    if _PROG is None:
        _PROG = build_program(N_CORES)
    in_maps = make_in_maps(**inputs)
    res = run_bass_kernel_spmd(_PROG, in_maps, core_ids=list(range(N_CORES)))
    return assemble_output(res.results)
